# revision 9
# baseline (speedup 1.0000x reference)
"""CRF forward-algorithm kernel for Trainium2 (8 NeuronCores, Bass).

Strategy: data-parallel over batch (32 -> 4 per core) PLUS chunked-scan
parallelism over time. The recursion

    alpha_t[b,j] = scores[b,t,j] + lse_i(trans[i,j] + alpha_{t-1}[b,i])

is run in linear space with a global per-step normalizer K:

    p_t[j,(c,b)] = exp(scores - K) * sum_i E[i,j] p_{t-1}[i,(c,b)]

The key observation: E = exp(trans) has entries in [e^-0.1, e^0.1], so the
linear map contracts the Hilbert projective metric by ~0.1 per step. T=512
is split into C=28 chunks of L=18 steps; every chunk runs concurrently as
extra matmul columns, warm-started W=8 steps early from a surrogate init
(alpha ~ emission scores). After W steps the state DIRECTION matches the
true chain to ~1e-8; only a per-chunk scalar log-offset delta remains,
recovered by a sequential stitch (prefix-sum over chunk boundary
mismatches, done as one small triangular matmul) and added to the output.
Sequential scan length drops 512 -> S = L+W = 26 steps.

Per step: one PE matmul (E stationary bf16, 112 moving columns) + one DVE
multiply. ln/transposes/Kt-correction pipeline behind the scan on ACT/PE/
DVE; the output leaves in a [(chunk,batch) partition, (step,tag)] layout
where the delta correction is a native per-partition tensor_scalar add,
and K*t - 10000*[j==0] is a host-precomputed constant fused into the
PSUM->SBUF copy. Host does only layout permutes (gather/scatter), no math.
"""

import numpy as np

N = 64
T = 512
B = 32
NCORES = 8
BS = B // NCORES   # 4 batch elements per core
C = 28             # time chunks
W = 8              # warmup steps per chunk
L = (T - W) // C   # 18 real steps per chunk (chunk 0: L+W)
S = L + W          # 26 sequential scan steps
CB = C * BS        # 112 columns per scan step
NCOL = S * CB      # 2912 state columns
K = 4.66
EPIECE = 8         # exp/DMA pieces for the score tensor
EC = NCOL // EPIECE  # 364 cols per piece
FPIECE = 4         # final delta-add/DMA-out pieces
FC = S * N // FPIECE  # 416 cols per piece


def _piece(r):
    # exp piece needed before scan column-slice r is read
    return ((r + 1) * CB + EC - 1) // EC


def _build_program():
    import concourse.bass as bass
    import concourse.mybir as mybir

    FT = mybir.dt.float32
    BF = mybir.dt.bfloat16
    AF = mybir.ActivationFunctionType

    nc = bass.Bass()
    scp_d = nc.declare_dram_parameter("scp", [N, NCOL], FT, isOutput=False)
    tr_d = nc.declare_dram_parameter("tr", [N, N], FT, isOutput=False)
    tcol_d = nc.declare_dram_parameter("tcol", [N, 1], FT, isOutput=False)
    kc_d = nc.declare_dram_parameter("kconst", [N, 2], FT, isOutput=False)
    id_d = nc.declare_dram_parameter("ident", [N, N], FT, isOutput=False)
    lm_d = nc.declare_dram_parameter("lmaskT", [CB, CB], FT, isOutput=False)
    ktf_d = nc.declare_dram_parameter("ktfull", [CB, S * N], FT, isOutput=False)
    out_d = nc.declare_dram_parameter("out", [CB, S * N], FT, isOutput=True)

    from contextlib import ExitStack

    with ExitStack() as ctx:
        es_sc = ctx.enter_context(nc.sbuf_tensor([N, NCOL], FT))
        es = ctx.enter_context(nc.sbuf_tensor([N, NCOL], FT))
        p_all = ctx.enter_context(nc.sbuf_tensor([N, NCOL], BF))
        lnp = ctx.enter_context(nc.sbuf_tensor([N, NCOL], FT))
        e_sb = ctx.enter_context(nc.sbuf_tensor([N, N], BF))
        tr_nat = ctx.enter_context(nc.sbuf_tensor([N, N], FT))
        tcol_sb = ctx.enter_context(nc.sbuf_tensor([N, 1], FT))
        e0k = ctx.enter_context(nc.sbuf_tensor([N, 1], FT))
        kc_sb = ctx.enter_context(nc.sbuf_tensor([N, 2], FT))
        ident = ctx.enter_context(nc.sbuf_tensor([N, N], FT))
        lm_sb = ctx.enter_context(nc.sbuf_tensor([CB, CB], FT))
        ktf_sb = ctx.enter_context(nc.sbuf_tensor([CB, S * N], FT))
        out_tr = ctx.enter_context(nc.sbuf_tensor([CB, S * N], FT))
        vfull = ctx.enter_context(nc.sbuf_tensor([N, CB], FT))
        vT_sb = ctx.enter_context(nc.sbuf_tensor([CB, N], FT))
        d_sb = ctx.enter_context(nc.sbuf_tensor([CB, 1], FT))
        s_ps = ctx.enter_context(nc.psum_tensor([N, CB], FT))
        tq0 = ctx.enter_context(nc.psum_tensor([CB, 2 * N], FT))
        tq1 = ctx.enter_context(nc.psum_tensor([CB, 2 * N], FT))
        tq2 = ctx.enter_context(nc.psum_tensor([CB, 2 * N], FT))
        tq3 = ctx.enter_context(nc.psum_tensor([CB, 2 * N], FT))
        vt_ps = ctx.enter_context(nc.psum_tensor([CB, N], FT))
        d_ps = ctx.enter_context(nc.psum_tensor([CB, N], FT))
        dma_c = ctx.enter_context(nc.semaphore())
        dma_es = ctx.enter_context(nc.semaphore())
        dma_kt = ctx.enter_context(nc.semaphore())
        acte = ctx.enter_context(nc.semaphore())
        exp_sem = ctx.enter_context(nc.semaphore())
        mset = ctx.enter_context(nc.semaphore())
        dve = ctx.enter_context(nc.semaphore())
        pe = ctx.enter_context(nc.semaphore())
        ln_sem = ctx.enter_context(nc.semaphore())
        tp_sem = ctx.enter_context(nc.semaphore())
        vsub = ctx.enter_context(nc.semaphore())
        tpv = ctx.enter_context(nc.semaphore())
        vtc = ctx.enter_context(nc.semaphore())
        dmm = ctx.enter_context(nc.semaphore())
        dsb = ctx.enter_context(nc.semaphore())
        fin = ctx.enter_context(nc.semaphore())
        outd = ctx.enter_context(nc.semaphore())
        block = ctx.enter_context(nc.Block())
        tq = [tq0, tq1, tq2, tq3]

        @block.sync
        def _(sync):
            def es_piece(k):
                sync.dma_start(
                    es_sc[:, k * EC : (k + 1) * EC], scp_d[:, k * EC : (k + 1) * EC]
                ).then_inc(dma_es, 16)

            es_piece(0)
            sync.dma_start(tr_nat[:, :], tr_d[:, :]).then_inc(dma_c, 16)
            sync.dma_start(tcol_sb[:, :], tcol_d[:, :]).then_inc(dma_c, 16)
            sync.dma_start(kc_sb[:, :], kc_d[:, :]).then_inc(dma_c, 16)
            for k in range(1, EPIECE):
                es_piece(k)
            sync.dma_start(ident[:, :], id_d[:, :]).then_inc(dma_c, 16)
            sync.dma_start(lm_sb[:, :], lm_d[:, :]).then_inc(dma_c, 16)
            sync.dma_start(ktf_sb[:, :], ktf_d[:, :]).then_inc(dma_kt, 16)
            for k in range(FPIECE):
                sync.wait_ge(fin, k + 1)
                sync.dma_start(
                    out_d[:, k * FC : (k + 1) * FC], out_tr[:, k * FC : (k + 1) * FC]
                ).then_inc(outd, 16)

        def tp_op(tensor, rp):
            # tq bank reuse (h vs h-4) is safe without a wait: the scan
            # matmul before this transpose waited dve>=2h+4, and tqa_op(h-3)
            # precedes mul_{2h+3} in the in-order DVE program.
            h = rp // 2
            t = tensor.transpose(
                tq[h % 4][:, (rp % 2) * N : (rp % 2 + 1) * N],
                lnp[:, rp * CB : (rp + 1) * CB],
                ident[:, :],
            )
            t._wait_ge(ln_sem, h + 1)
            t.then_inc(tp_sem, 1)

        @block.tensor
        def _(tensor):
            tensor.wait_ge(mset, 1)
            for r in range(1, S):
                mm = tensor.matmul(
                    s_ps[:, :], e_sb[:, :], p_all[:, (r - 1) * CB : r * CB]
                )
                mm._wait_ge(dve, r)
                mm.then_inc(pe, 1)
                if r == 4:
                    tensor.wait_ge(dma_c, 64)
                if r >= 4:
                    tp_op(tensor, r - 4)
            for rp in range(S - 4, S):
                tp_op(tensor, rp)
            tv = tensor.transpose(vt_ps[:, :], vfull[:, :], ident[:, :])
            tv._wait_ge(vsub, 1)
            tv.then_inc(tpv, 1)
            dm = tensor.matmul(d_ps[:, :], lm_sb[:, :], vT_sb[:, :])
            dm._wait_ge(vtc, 1)
            dm.then_inc(dmm, 1)

        @block.scalar
        def _(scalar):
            scalar.wait_ge(dma_c, 48)
            scalar.wait_ge(dma_es, 16)
            scalar.activation(
                es[:, 0:EC], es_sc[:, 0:EC], AF.Exp, bias=kc_sb[:, 1:2]
            ).then_inc(exp_sem, 1)
            scalar.activation(e_sb[:, :], tr_nat[:, :], AF.Exp).then_inc(acte, 1)
            scalar.activation(
                e0k[:, :], tcol_sb[:, :], AF.Exp, bias=kc_sb[:, 0:1]
            ).then_inc(acte, 1)
            for k in range(1, EPIECE):
                scalar.wait_ge(dma_es, 16 * (k + 1))
                scalar.activation(
                    es[:, k * EC : (k + 1) * EC],
                    es_sc[:, k * EC : (k + 1) * EC],
                    AF.Exp,
                    bias=kc_sb[:, 1:2],
                ).then_inc(exp_sem, 1)
            for h in range(S // 2):
                a = scalar.activation(
                    lnp[:, 2 * h * CB : (2 * h + 2) * CB],
                    p_all[:, 2 * h * CB : (2 * h + 2) * CB],
                    AF.Ln,
                )
                a._wait_ge(dve, 2 * h + 2)
                a.then_inc(ln_sem, 1)
            cp1 = scalar.copy(vT_sb[:, :], vt_ps[:, :])
            cp1._wait_ge(tpv, 1)
            cp1.then_inc(vtc, 1)
            cp2 = scalar.copy(d_sb[:, :], d_ps[:, 1:2])
            cp2._wait_ge(dmm, 1)
            cp2.then_inc(dsb, 1)

        def tqa_op(vector, h):
            a = vector.tensor_add(
                out_tr[:, h * 2 * N : (h + 1) * 2 * N],
                tq[h % 4][:, :],
                ktf_sb[:, h * 2 * N : (h + 1) * 2 * N],
            )
            a._wait_ge(tp_sem, 2 * h + 2)

        @block.vector
        def _(vector):
            vector.wait_ge(acte, 1)
            vector.memset(e_sb[:, 0:1], 1.0)
            vector.memset(e_sb[0:1, :], 0.0).then_inc(mset, 1)
            vector.wait_ge(acte, 2)
            vector.memset(e0k[0:1, 0:1], float(np.exp(K)))
            vector.memset(vfull[:, 0:BS], 0.0)
            vector.wait_ge(exp_sem, 1)
            vector.tensor_scalar_mul(
                p_all[:, 0:CB], es[:, 0:CB], e0k[:, :]
            ).then_inc(dve, 1)
            for r in range(1, S):
                if _piece(r) > _piece(r - 1):
                    vector.wait_ge(exp_sem, _piece(r))
                m = vector.tensor_mul(
                    p_all[:, r * CB : (r + 1) * CB],
                    s_ps[:, :],
                    es[:, r * CB : (r + 1) * CB],
                )
                m._wait_ge(pe, r)
                m.then_inc(dve, 1)
                if r == 5:
                    vector.wait_ge(dma_kt, 16)
                if r >= 5 and (r - 5) % 2 == 0:
                    tqa_op(vector, (r - 5) // 2)
            for h in range((S - 5) // 2 + 1, S // 2):
                tqa_op(vector, h)
            sub = vector.tensor_sub(
                vfull[:, BS:CB],
                lnp[:, (S - 1) * CB : (S - 1) * CB + (C - 1) * BS],
                lnp[:, (W - 1) * CB + BS : (W - 1) * CB + CB],
            )
            sub._wait_ge(ln_sem, S // 2)
            sub.then_inc(vsub, 1)
            for k in range(FPIECE):
                f = vector.tensor_scalar_add(
                    out_tr[:, k * FC : (k + 1) * FC],
                    out_tr[:, k * FC : (k + 1) * FC],
                    d_sb[:, :],
                )
                if k == 0:
                    f._wait_ge(dsb, 1)
                f.then_inc(fin, 1)

    return nc


LAST_RESULT = None


def kernel(scores: np.ndarray, transitions: np.ndarray) -> np.ndarray:
    global LAST_RESULT
    from concourse.bass_utils import run_bass_kernel_spmd

    scores = np.ascontiguousarray(scores, dtype=np.float32)
    transitions = np.ascontiguousarray(transitions, dtype=np.float32)

    # host-side constants and layout permutes (no math on the data path)
    idx_t = np.arange(C)[None, :] * L + np.arange(S)[:, None]      # (S, C)
    tcol = np.ascontiguousarray(transitions[0, :].reshape(N, 1))
    kconst = np.stack(
        [np.full(N, K, np.float32), np.full(N, -K, np.float32)], axis=1
    )
    ident = np.eye(N, dtype=np.float32)
    cidx = np.repeat(np.arange(C), BS)
    bidx = np.tile(np.arange(BS), C)
    M = (
        (bidx[:, None] == bidx[None, :])
        & (cidx[None, :] >= 1)
        & (cidx[None, :] <= cidx[:, None])
    ).astype(np.float32)
    lmaskT = np.ascontiguousarray(M.T)
    tvals = (np.arange(C)[:, None] * L + np.arange(S)[None, :]).astype(np.float32)
    ktf = np.repeat(K * tvals[:, None, :], BS, axis=1).reshape(CB, S)
    ktfull = np.repeat(ktf[:, :, None], N, axis=2).reshape(CB, S * N)
    ktfull[:, 0::N] -= 10000.0
    ktfull = np.ascontiguousarray(ktfull)

    nc = _build_program()
    in_maps = []
    for g in range(NCORES):
        blk = scores[g * BS : (g + 1) * BS]                 # (BS, T, N)
        scp = np.ascontiguousarray(
            blk[:, idx_t, :].transpose(3, 1, 2, 0).reshape(N, NCOL)
        )
        in_maps.append(
            {"scp": scp, "tr": transitions, "tcol": tcol, "kconst": kconst,
             "ident": ident, "lmaskT": lmaskT, "ktfull": ktfull}
        )
    res = run_bass_kernel_spmd(nc, in_maps, list(range(NCORES)))
    LAST_RESULT = res
    out = np.empty((B, T, N), dtype=np.float32)
    for g in range(NCORES):
        arr = res.results[g]["out"].reshape(C, BS, S, N)
        og = out[g * BS : (g + 1) * BS]
        og[:, 0:S] = arr[0]
        for c in range(1, C):
            og[:, c * L + W : c * L + S] = arr[c, :, W:S]
    return out


# revision 10
# speedup vs baseline: 1.1107x; 1.1107x over previous
"""CRF forward-algorithm kernel for Trainium2 (8 NeuronCores, Bass).

Strategy: data-parallel over batch (32 -> 4 per core) PLUS chunked-scan
parallelism over time. The recursion

    alpha_t[b,j] = scores[b,t,j] + lse_i(trans[i,j] + alpha_{t-1}[b,i])

is run in linear space with a global per-step normalizer K:

    p_t[j,(c,b)] = exp(scores - K) * sum_i E[i,j] p_{t-1}[i,(c,b)]

The key observation: E = exp(trans) has entries in [e^-0.1, e^0.1], so the
linear map contracts the Hilbert projective metric by ~0.1 per step. T=512
is split into C=28 chunks of L=18 steps; every chunk runs concurrently as
extra matmul columns, warm-started W=8 steps early from a surrogate init
(alpha ~ emission scores). After W steps the state DIRECTION matches the
true chain to ~1e-8; only a per-chunk scalar log-offset delta remains,
recovered by a sequential stitch (prefix-sum over chunk boundary
mismatches, done as one small triangular matmul) and added to the output.
Sequential scan length drops 512 -> S = L+W = 26 steps.

Per step: one PE matmul (E stationary bf16, 112 moving columns) + one DVE
multiply. ln/transposes/Kt-correction pipeline behind the scan on ACT/PE/
DVE; the output leaves in a [(chunk,batch) partition, (step,tag)] layout
where the delta correction is a native per-partition tensor_scalar add,
and K*t - 10000*[j==0] is a host-precomputed constant fused into the
PSUM->SBUF copy. Host does only layout permutes (gather/scatter), no math.
"""

import numpy as np

N = 64
T = 512
B = 32
NCORES = 8
BS = B // NCORES   # 4 batch elements per core
C = 28             # time chunks
W = 8              # warmup steps per chunk
L = (T - W) // C   # 18 real steps per chunk (chunk 0: L+W)
S = L + W          # 26 sequential scan steps
CB = C * BS        # 112 columns per scan step
NCOL = S * CB      # 2912 state columns
K = 4.66
EPIECES = [112, 336, 1232, 1232]   # asymmetric exp/DMA pieces (cols)
ECUM = [sum(EPIECES[: i + 1]) for i in range(len(EPIECES))]  # cumulative
FPIECE = 2         # final delta-add/DMA-out pieces
FC = S * N // FPIECE  # 832 cols per piece


def _piece(r):
    # exp pieces needed before scan column-slice r is read
    need = (r + 1) * CB
    for i, c in enumerate(ECUM):
        if c >= need:
            return i + 1
    return len(ECUM)


def _build_program():
    import concourse.bass as bass
    import concourse.mybir as mybir

    FT = mybir.dt.float32
    BF = mybir.dt.bfloat16
    AF = mybir.ActivationFunctionType

    nc = bass.Bass()
    scp_d = nc.declare_dram_parameter("scp", [N, NCOL], FT, isOutput=False)
    cst_d = nc.declare_dram_parameter("consts", [CB, 2 * N + 3 + CB], FT,
                                      isOutput=False)
    ktf_d = nc.declare_dram_parameter("ktfull", [CB, S * N], FT, isOutput=False)
    out_d = nc.declare_dram_parameter("out", [CB, S * N], FT, isOutput=True)

    from contextlib import ExitStack

    with ExitStack() as ctx:
        es_sc = ctx.enter_context(nc.sbuf_tensor([N, NCOL], FT))
        es = ctx.enter_context(nc.sbuf_tensor([N, NCOL], FT))
        p_all = ctx.enter_context(nc.sbuf_tensor([N, NCOL], BF))
        lnp = ctx.enter_context(nc.sbuf_tensor([N, NCOL], FT))
        e_sb = ctx.enter_context(nc.sbuf_tensor([N, N], BF))
        cst = ctx.enter_context(nc.sbuf_tensor([CB, 2 * N + 3 + CB], FT))
        e0k = ctx.enter_context(nc.sbuf_tensor([N, 1], FT))
        tr_nat = cst[0:N, 0:N]
        tcol_sb = cst[0:N, N : N + 1]
        kc_sb = cst[0:N, N + 1 : N + 3]
        ident = cst[0:N, N + 3 : 2 * N + 3]
        lm_sb = cst[:, 2 * N + 3 : 2 * N + 3 + CB]
        ktf_sb = ctx.enter_context(nc.sbuf_tensor([CB, S * N], FT))
        out_tr = ctx.enter_context(nc.sbuf_tensor([CB, S * N], FT))
        vfull = ctx.enter_context(nc.sbuf_tensor([N, CB], FT))
        vT_sb = ctx.enter_context(nc.sbuf_tensor([CB, N], FT))
        d_sb = ctx.enter_context(nc.sbuf_tensor([CB, 1], FT))
        s_ps = ctx.enter_context(nc.psum_tensor([N, CB], FT))
        tq0 = ctx.enter_context(nc.psum_tensor([CB, 2 * N], FT))
        tq1 = ctx.enter_context(nc.psum_tensor([CB, 2 * N], FT))
        tq2 = ctx.enter_context(nc.psum_tensor([CB, 2 * N], FT))
        tq3 = ctx.enter_context(nc.psum_tensor([CB, 2 * N], FT))
        vt_ps = ctx.enter_context(nc.psum_tensor([CB, N], FT))
        d_ps = ctx.enter_context(nc.psum_tensor([CB, N], FT))
        dma_c = ctx.enter_context(nc.semaphore())
        dma_es = ctx.enter_context(nc.semaphore())
        dma_kt = ctx.enter_context(nc.semaphore())
        acte = ctx.enter_context(nc.semaphore())
        exp_sem = ctx.enter_context(nc.semaphore())
        mset = ctx.enter_context(nc.semaphore())
        dve = ctx.enter_context(nc.semaphore())
        pe = ctx.enter_context(nc.semaphore())
        ln_sem = ctx.enter_context(nc.semaphore())
        tp_sem = ctx.enter_context(nc.semaphore())
        vsub = ctx.enter_context(nc.semaphore())
        tpv = ctx.enter_context(nc.semaphore())
        vtc = ctx.enter_context(nc.semaphore())
        dmm = ctx.enter_context(nc.semaphore())
        dsb = ctx.enter_context(nc.semaphore())
        fin = ctx.enter_context(nc.semaphore())
        outd = ctx.enter_context(nc.semaphore())
        block = ctx.enter_context(nc.Block())
        tq = [tq0, tq1, tq2, tq3]

        @block.sync
        def _(sync):
            sync.dma_start(cst[:, :], cst_d[:, :]).then_inc(dma_c, 16)
            for k in range(len(EPIECES)):
                lo = ECUM[k] - EPIECES[k]
                sync.dma_start(
                    es_sc[:, lo : ECUM[k]], scp_d[:, lo : ECUM[k]]
                ).then_inc(dma_es, 16)
            sync.dma_start(ktf_sb[:, :], ktf_d[:, :]).then_inc(dma_kt, 16)
            for k in range(FPIECE):
                sync.wait_ge(fin, k + 1)
                sync.dma_start(
                    out_d[:, k * FC : (k + 1) * FC], out_tr[:, k * FC : (k + 1) * FC]
                ).then_inc(outd, 16)

        def tp_op(tensor, rp):
            # tq bank reuse (h vs h-4) is safe without a wait: the scan
            # matmul before this transpose waited dve>=2h+4, and tqa_op(h-3)
            # precedes mul_{2h+3} in the in-order DVE program.
            h = rp // 2
            t = tensor.transpose(
                tq[h % 4][:, (rp % 2) * N : (rp % 2 + 1) * N],
                lnp[:, rp * CB : (rp + 1) * CB],
                ident[:, :],
            )
            t._wait_ge(ln_sem, h + 1)
            t.then_inc(tp_sem, 1)

        @block.tensor
        def _(tensor):
            tensor.wait_ge(mset, 1)
            for r in range(1, S):
                mm = tensor.matmul(
                    s_ps[:, :], e_sb[:, :], p_all[:, (r - 1) * CB : r * CB]
                )
                mm._wait_ge(dve, r)
                mm.then_inc(pe, 1)
                if r == 4:
                    tensor.wait_ge(dma_c, 16)
                if r >= 4:
                    tp_op(tensor, r - 4)
            for rp in range(S - 4, S):
                tp_op(tensor, rp)
            tv = tensor.transpose(vt_ps[:, :], vfull[:, :], ident[:, :])
            tv._wait_ge(vsub, 1)
            tv.then_inc(tpv, 1)
            dm = tensor.matmul(d_ps[:, :], lm_sb[:, :], vT_sb[:, :])
            dm._wait_ge(vtc, 1)
            dm.then_inc(dmm, 1)

        @block.scalar
        def _(scalar):
            scalar.wait_ge(dma_c, 16)
            scalar.activation(e_sb[:, :], tr_nat[:, :], AF.Exp).then_inc(acte, 1)
            scalar.activation(
                e0k[:, :], tcol_sb[:, :], AF.Exp, bias=kc_sb[:, 0:1]
            ).then_inc(acte, 1)
            for k in range(len(EPIECES)):
                lo = ECUM[k] - EPIECES[k]
                scalar.wait_ge(dma_es, 16 * (k + 1))
                scalar.activation(
                    es[:, lo : ECUM[k]],
                    es_sc[:, lo : ECUM[k]],
                    AF.Exp,
                    bias=kc_sb[:, 1:2],
                ).then_inc(exp_sem, 1)
            for h in range(S // 2):
                a = scalar.activation(
                    lnp[:, 2 * h * CB : (2 * h + 2) * CB],
                    p_all[:, 2 * h * CB : (2 * h + 2) * CB],
                    AF.Ln,
                )
                a._wait_ge(dve, 2 * h + 2)
                a.then_inc(ln_sem, 1)
            cp1 = scalar.copy(vT_sb[:, :], vt_ps[:, :])
            cp1._wait_ge(tpv, 1)
            cp1.then_inc(vtc, 1)
            cp2 = scalar.copy(d_sb[:, :], d_ps[:, 1:2])
            cp2._wait_ge(dmm, 1)
            cp2.then_inc(dsb, 1)

        def tqa_op(vector, h):
            a = vector.tensor_add(
                out_tr[:, h * 2 * N : (h + 1) * 2 * N],
                tq[h % 4][:, :],
                ktf_sb[:, h * 2 * N : (h + 1) * 2 * N],
            )
            a._wait_ge(tp_sem, 2 * h + 2)

        @block.vector
        def _(vector):
            vector.wait_ge(acte, 1)
            vector.memset(e_sb[:, 0:1], 1.0)
            vector.memset(e_sb[0:1, :], 0.0).then_inc(mset, 1)
            vector.wait_ge(acte, 2)
            vector.memset(e0k[0:1, 0:1], float(np.exp(K)))
            vector.memset(vfull[:, 0:BS], 0.0)
            vector.wait_ge(exp_sem, 1)
            vector.tensor_scalar_mul(
                p_all[:, 0:CB], es[:, 0:CB], e0k[:, :]
            ).then_inc(dve, 1)
            for r in range(1, S):
                if _piece(r) > _piece(r - 1):
                    vector.wait_ge(exp_sem, _piece(r))
                m = vector.tensor_mul(
                    p_all[:, r * CB : (r + 1) * CB],
                    s_ps[:, :],
                    es[:, r * CB : (r + 1) * CB],
                )
                m._wait_ge(pe, r)
                m.then_inc(dve, 1)
                if r == 5:
                    vector.wait_ge(dma_kt, 16)
                if r >= 5 and (r - 5) % 2 == 0:
                    tqa_op(vector, (r - 5) // 2)
            for h in range((S - 5) // 2 + 1, S // 2):
                tqa_op(vector, h)
            sub = vector.tensor_sub(
                vfull[:, BS:CB],
                lnp[:, (S - 1) * CB : (S - 1) * CB + (C - 1) * BS],
                lnp[:, (W - 1) * CB + BS : (W - 1) * CB + CB],
            )
            sub._wait_ge(ln_sem, S // 2)
            sub.then_inc(vsub, 1)
            for k in range(FPIECE):
                f = vector.tensor_scalar_add(
                    out_tr[:, k * FC : (k + 1) * FC],
                    out_tr[:, k * FC : (k + 1) * FC],
                    d_sb[:, :],
                )
                if k == 0:
                    f._wait_ge(dsb, 1)
                f.then_inc(fin, 1)

    return nc


LAST_RESULT = None


def kernel(scores: np.ndarray, transitions: np.ndarray) -> np.ndarray:
    global LAST_RESULT
    from concourse.bass_utils import run_bass_kernel_spmd

    scores = np.ascontiguousarray(scores, dtype=np.float32)
    transitions = np.ascontiguousarray(transitions, dtype=np.float32)

    # host-side constants and layout permutes (no math on the data path)
    idx_t = np.arange(C)[None, :] * L + np.arange(S)[:, None]      # (S, C)
    consts = np.zeros((CB, 2 * N + 3 + CB), np.float32)
    consts[0:N, 0:N] = transitions
    consts[0:N, N] = transitions[0, :]
    consts[0:N, N + 1] = K
    consts[0:N, N + 2] = -K
    consts[0:N, N + 3 : 2 * N + 3] = np.eye(N, dtype=np.float32)
    cidx = np.repeat(np.arange(C), BS)
    bidx = np.tile(np.arange(BS), C)
    M = (
        (bidx[:, None] == bidx[None, :])
        & (cidx[None, :] >= 1)
        & (cidx[None, :] <= cidx[:, None])
    ).astype(np.float32)
    consts[:, 2 * N + 3 :] = M.T
    tvals = (np.arange(C)[:, None] * L + np.arange(S)[None, :]).astype(np.float32)
    ktf = np.repeat(K * tvals[:, None, :], BS, axis=1).reshape(CB, S)
    ktfull = np.repeat(ktf[:, :, None], N, axis=2).reshape(CB, S * N)
    ktfull[:, 0::N] -= 10000.0
    ktfull = np.ascontiguousarray(ktfull)

    nc = _build_program()
    in_maps = []
    for g in range(NCORES):
        blk = scores[g * BS : (g + 1) * BS]                 # (BS, T, N)
        scp = np.ascontiguousarray(
            blk[:, idx_t, :].transpose(3, 1, 2, 0).reshape(N, NCOL)
        )
        in_maps.append(
            {"scp": scp, "consts": consts, "ktfull": ktfull}
        )
    res = run_bass_kernel_spmd(nc, in_maps, list(range(NCORES)))
    LAST_RESULT = res
    out = np.empty((B, T, N), dtype=np.float32)
    for g in range(NCORES):
        arr = res.results[g]["out"].reshape(C, BS, S, N)
        og = out[g * BS : (g + 1) * BS]
        og[:, 0:S] = arr[0]
        for c in range(1, C):
            og[:, c * L + W : c * L + S] = arr[c, :, W:S]
    return out
